# revision 2
# baseline (speedup 1.0000x reference)
"""Trainium2 Bass kernel for the label-attention decoder.

Computation (per reference):
    z      = tanh(x @ W_w.T + W_b)            [B, S, D_A]
    scores = z @ U_w.T  (per label)           [B, L, S]
    alpha  = softmax(scores, axis=S)          [B, L, S]   (output)
    m      = alpha @ z                        [B, L, D_A]
    y      = rowwise_dot(V_w, m) + V_b        [B, L]      (output)

Sharding: labels L are split across 8 cores (L padded 8921 -> 9216 = 8*1152).
Each core recomputes z for the full batch (x is replicated), then handles its
1152-label slice fully independently; outputs are concatenated on the host.

Key layout facts:
  - PE matmul computes lhsT.T @ rhs, contracting over the partition dim.
  - z is kept on-chip in BOTH layouts per batch: z_T [a, s] (for scores) and
    z_nat [s, a] (for m).  All transposes (x during load, z_T->z_nat,
    exp->alpha_T) run on the DMA xbar (bf16), keeping the PE for matmuls only.
  - Softmax is computed without max-subtraction: scores = U.z with U rows
    scaled 1/sqrt(D_A) and z in (-1,1), so |scores| < ~6 and exp() is safe.
  - S is padded 4000 -> 4096; the pad columns of the scores PSUM are
    overwritten with -30 before exp(), making pad contributions ~1e-13.
  - The m matmul uses UNNORMALIZED exp (bf16); y is rescaled by 1/sum at the
    end (y = (V.m_unnorm) * recip + V_b).  alpha output is normalized f32.
"""

import numpy as np
import ml_dtypes

B, S, D_IN, D_A, L = 4, 4000, 1024, 512, 8921
N_CORES = 8
LC = 1152               # labels per core; 8 * 1152 = 9216 >= 8921
L_PAD = N_CORES * LC
LB = 128                # label block (PSUM partitions / xbar-transpose unit)
NLB = LC // LB          # 9
P = 128
SP = 4096               # padded sequence length
NS = SP // P            # 32 s-tiles of 128
NSC = SP // 512         # 8 s-chunks of 512
ND = D_IN // P          # 8 d-chunks
NA = D_A // P           # 4 a-chunks

_CACHE = {}


def _build_nc():
    import concourse.mybir as mybir
    import concourse.tile as tile
    from concourse import bacc

    f32 = mybir.dt.float32
    bf16 = mybir.dt.bfloat16
    Tanh = mybir.ActivationFunctionType.Tanh
    Exp = mybir.ActivationFunctionType.Exp
    AX = mybir.AxisListType.X
    Alu = mybir.AluOpType

    nc = bacc.Bacc("TRN2", target_bir_lowering=False, debug=False)

    x_d = nc.dram_tensor("x", (B, S, D_IN), bf16, kind="ExternalInput").ap()
    WT_d = nc.dram_tensor("WT", (D_IN, D_A), bf16, kind="ExternalInput").ap()
    Wb_d = nc.dram_tensor("Wb", (D_A,), f32, kind="ExternalInput").ap()
    UT_d = nc.dram_tensor("UT", (D_A, LC), bf16, kind="ExternalInput").ap()
    V_d = nc.dram_tensor("V", (LC, D_A), f32, kind="ExternalInput").ap()
    Vb_d = nc.dram_tensor("Vb", (LC,), f32, kind="ExternalInput").ap()
    alpha_d = nc.dram_tensor("alpha_out", (B, LC, S), f32, kind="ExternalOutput").ap()
    y_d = nc.dram_tensor("y_out", (B, LC), f32, kind="ExternalOutput").ap()

    with tile.TileContext(nc) as tc:
        with (
            tc.tile_pool(name="consts", bufs=1) as consts,
            tc.tile_pool(name="zpool", bufs=1) as zpool,
            tc.tile_pool(name="xw", bufs=2) as xw,
            tc.tile_pool(name="epool", bufs=2) as epool,
            tc.tile_pool(name="alph", bufs=2) as alph,
            tc.tile_pool(name="apool", bufs=2) as apool,
            tc.tile_pool(name="spool", bufs=4) as spool,
            tc.tile_pool(name="ypool", bufs=2) as ypool,
            tc.tile_pool(name="psum_z", bufs=3, space="PSUM") as psum_z,
            tc.tile_pool(name="psum_s", bufs=3, space="PSUM") as psum_s,
            tc.tile_pool(name="psum_m", bufs=2, space="PSUM") as psum_m,
        ):
            # ---- constants ----
            WT_s = consts.tile([P, ND, D_A], bf16)   # [d_in, dc, a]
            nc.sync.dma_start(WT_s[:], WT_d.rearrange("(nd p) a -> p nd a", p=P))
            UT_s = consts.tile([P, NA, LC], bf16)    # [a_in, ac, l]
            nc.sync.dma_start(UT_s[:], UT_d.rearrange("(na p) l -> p na l", p=P))
            V_s = consts.tile([LB, NLB, D_A], f32)   # [l_in, lc, a]
            nc.sync.dma_start(V_s[:], V_d.rearrange("(nl lb) a -> lb nl a", lb=LB))
            with nc.allow_non_contiguous_dma(reason="tiny 1-D bias loads"):
                Wb_s = consts.tile([P, NA], f32)
                nc.gpsimd.dma_start(Wb_s[:], Wb_d.rearrange("(na p) -> p na", p=P))
                Vb_s = consts.tile([LB, NLB], f32)
                nc.gpsimd.dma_start(Vb_s[:], Vb_d.rearrange("(nl lb) -> lb nl", lb=LB))

            for b in range(B):
                # ---------- phase Z: z_T[a, s] and z_nat[s, a] (bf16) ----------
                zT = zpool.tile([P, NA, SP], bf16, tag="zT")
                zN = zpool.tile([P, NS, D_A], bf16, tag="zN")
                for sc in range(NSC):
                    s0 = sc * 512
                    sv = min(512, S - s0)           # 512, or 416 for the last
                    xT = xw.tile([P, ND, 512], bf16)
                    for dc in range(ND):
                        nc.sync.dma_start(
                            out=xT[:, dc, :sv],
                            in_=x_d[b, s0 : s0 + sv, dc * P : (dc + 1) * P],
                            transpose=True,
                        )
                    if sv < 512:
                        nc.gpsimd.memset(xT[:, :, sv:], 0.0)
                    for ao in range(NA):
                        pz = psum_z.tile([P, 512], f32, tag="pz")
                        for dc in range(ND):
                            nc.tensor.matmul(
                                pz[:],
                                WT_s[:, dc, ao * P : (ao + 1) * P],
                                xT[:, dc, :],
                                start=(dc == 0),
                                stop=(dc == ND - 1),
                            )
                        nc.scalar.activation(
                            out=zT[:, ao, s0 : s0 + 512],
                            in_=pz[:],
                            func=Tanh,
                            bias=Wb_s[:, ao : ao + 1],
                        )
                # z_nat from z_T via DMA xbar transpose (bf16, [128,128] units)
                for ao in range(NA):
                    for so in range(NS):
                        nc.sync.dma_start(
                            out=zN[:, so, ao * P : (ao + 1) * P],
                            in_=zT[:, ao, so * P : (so + 1) * P],
                            transpose=True,
                        )

                # ---------- label phase ----------
                yb = ypool.tile([LB, NLB], f32, tag="yb")
                for lc in range(NLB):
                    l0 = lc * LB
                    expt = epool.tile([LB, SP], bf16)
                    sums = spool.tile([LB, NSC], f32)
                    for sc in range(NSC):
                        ps = psum_s.tile([LB, 512], f32, tag="ps")
                        for ao in range(NA):
                            nc.tensor.matmul(
                                ps[:],
                                UT_s[:, ao, l0 : l0 + LB],
                                zT[:, ao, sc * 512 : (sc + 1) * 512],
                                start=(ao == 0),
                                stop=(ao == NA - 1),
                            )
                        if sc == NSC - 1:
                            # pad columns s in [4000, 4096): exp(-30) ~ 1e-13
                            nc.vector.memset(ps[:, S - 7 * 512 :], -30.0)
                        nc.scalar.activation(
                            out=expt[:, sc * 512 : (sc + 1) * 512],
                            in_=ps[:],
                            func=Exp,
                            accum_out=sums[:, sc : sc + 1],
                        )
                    tot = spool.tile([LB, 1], f32)
                    nc.vector.reduce_sum(out=tot[:], in_=sums[:], axis=AX)
                    rec = spool.tile([LB, 1], f32)
                    nc.vector.reciprocal(rec[:], tot[:])
                    # normalized f32 alpha for output
                    alpt = alph.tile([LB, S], f32)
                    nc.vector.tensor_scalar_mul(alpt[:], expt[:, :S], rec[:])
                    nc.sync.dma_start(out=alpha_d[b, l0 : l0 + LB, :], in_=alpt[:])
                    # alpha_T (unnormalized exp, bf16) via DMA xbar transpose
                    aT = apool.tile([P, NS, LB], bf16)
                    for so in range(NS):
                        nc.scalar.dma_start(
                            out=aT[:, so, :],
                            in_=expt[:, so * P : (so + 1) * P],
                            transpose=True,
                        )
                    pm = psum_m.tile([LB, D_A], f32, tag="pm")
                    for so in range(NS):
                        nc.tensor.matmul(
                            pm[:],
                            aT[:, so, :],
                            zN[:, so, :],
                            start=(so == 0),
                            stop=(so == NS - 1),
                        )
                    ytmp = ypool.tile([LB, D_A], f32, tag="ytmp")
                    nc.vector.tensor_mul(ytmp[:], V_s[:, lc, :], pm[:])
                    ysc = spool.tile([LB, 1], f32)
                    nc.vector.reduce_sum(out=ysc[:], in_=ytmp[:], axis=AX)
                    # y = y_unnorm * recip + V_b
                    nc.vector.tensor_scalar(
                        yb[:, lc : lc + 1],
                        ysc[:],
                        rec[:],
                        Vb_s[:, lc : lc + 1],
                        op0=Alu.mult,
                        op1=Alu.add,
                    )
                with nc.allow_non_contiguous_dma(reason="tiny y store"):
                    nc.gpsimd.dma_start(
                        out=y_d[b].rearrange("(nl lb) -> lb nl", lb=LB), in_=yb[:]
                    )

    nc.compile()
    return nc


def _get_nc():
    if "nc" not in _CACHE:
        _CACHE["nc"] = _build_nc()
    return _CACHE["nc"]


def _make_in_maps(x, W_w, W_b, U_w, V_w, V_b):
    bf = ml_dtypes.bfloat16
    x_bf = np.ascontiguousarray(x, dtype=np.float32).astype(bf)
    WT = np.ascontiguousarray(np.asarray(W_w, np.float32).T).astype(bf)
    Wb = np.ascontiguousarray(W_b, dtype=np.float32)
    U_pad = np.zeros((L_PAD, D_A), np.float32)
    U_pad[:L] = U_w
    UT = np.ascontiguousarray(U_pad.T).astype(bf)          # [D_A, L_PAD]
    V_pad = np.zeros((L_PAD, D_A), np.float32)
    V_pad[:L] = V_w
    Vb_pad = np.zeros((L_PAD,), np.float32)
    Vb_pad[:L] = V_b
    in_maps = []
    for c in range(N_CORES):
        sl = slice(c * LC, (c + 1) * LC)
        in_maps.append(
            {
                "x": x_bf,
                "WT": WT,
                "Wb": Wb,
                "UT": np.ascontiguousarray(UT[:, sl]),
                "V": np.ascontiguousarray(V_pad[sl]),
                "Vb": np.ascontiguousarray(Vb_pad[sl]),
            }
        )
    return in_maps


def run(inputs, trace=False, trace_cores=None):
    """Run on 8 cores; returns ((y, alpha), BassKernelResults)."""
    from concourse.bass_utils import run_bass_kernel_spmd

    nc = _get_nc()
    in_maps = _make_in_maps(**inputs)
    res = run_bass_kernel_spmd(
        nc,
        in_maps,
        core_ids=list(range(N_CORES)),
        trace=trace,
        trace_cores=trace_cores,
    )
    y = np.concatenate([r["y_out"] for r in res.results], axis=1)[:, :L]
    alpha = np.concatenate([r["alpha_out"] for r in res.results], axis=1)[:, :L, :]
    return (np.ascontiguousarray(y), np.ascontiguousarray(alpha)), res


def kernel(x, W_w, W_b, U_w, V_w, V_b):
    (y, alpha), _ = run(
        dict(x=x, W_w=W_w, W_b=W_b, U_w=U_w, V_w=V_w, V_b=V_b), trace=False
    )
    return y, alpha


# revision 3
# speedup vs baseline: 2.6769x; 2.6769x over previous
"""Trainium2 Bass kernel for the label-attention decoder.

Computation (per reference):
    z      = tanh(x @ W_w.T + W_b)            [B, S, D_A]
    scores = z @ U_w.T  (per label)           [B, L, S]
    alpha  = softmax(scores, axis=S)          [B, L, S]   (output)
    m      = alpha @ z                        [B, L, D_A]
    y      = rowwise_dot(V_w, m) + V_b        [B, L]      (output)

Sharding: labels L are split across 8 cores (L padded 8921 -> 9216 = 8*1152).
Each core recomputes z for the full batch (x is replicated), then handles its
1152-label slice fully independently; outputs are concatenated on the host.

Key layout facts:
  - PE matmul computes lhsT.T @ rhs, contracting over the partition dim.
  - z is kept on-chip in BOTH layouts per batch: z_T [a, s] (for scores) and
    z_nat [s, a] (for m).  x is transposed during its DRAM->SBUF DMA (xbar,
    bf16); z_nat and alpha_T are produced by PE transposes (SBUF->SBUF xbar
    transpose is both slow and corrupt for these shapes on TRN2 HW).
  - Softmax is computed without max-subtraction: scores = U.z with U rows
    scaled 1/sqrt(D_A) and z in (-1,1), so |scores| < ~6 and exp() is safe.
  - S is padded 4000 -> 4096; the pad columns of the scores PSUM are
    overwritten with -30 before exp(), making pad contributions ~1e-13.
  - Critical-path trick: alpha_T tiles are transposed from UNNORMALIZED exp
    as soon as each 512-column chunk is ready, so the m matmul never waits on
    the softmax reduction.  y is rescaled by 1/sum at the end; the normalized
    f32 alpha output (in-place scale + DMA) happens off the critical path.
"""

import numpy as np
import ml_dtypes

B, S, D_IN, D_A, L = 4, 4000, 1024, 512, 8921
N_CORES = 8
LC = 1152               # labels per core; 8 * 1152 = 9216 >= 8921
L_PAD = N_CORES * LC
LB = 128                # label block (PSUM partitions)
NLB = LC // LB          # 9
P = 128
SP = 4096               # padded sequence length
NS = SP // P            # 32 s-tiles of 128
NSC = SP // 512         # 8 s-chunks of 512
ND = D_IN // P          # 8 d-chunks
NA = D_A // P           # 4 a-chunks

_CACHE = {}


def _build_nc():
    import concourse.mybir as mybir
    import concourse.tile as tile
    from concourse import bacc
    from concourse.masks import make_identity

    f32 = mybir.dt.float32
    bf16 = mybir.dt.bfloat16
    Tanh = mybir.ActivationFunctionType.Tanh
    Exp = mybir.ActivationFunctionType.Exp
    AX = mybir.AxisListType.X
    Alu = mybir.AluOpType

    nc = bacc.Bacc("TRN2", target_bir_lowering=False, debug=False)

    x_d = nc.dram_tensor("x", (B, S, D_IN), bf16, kind="ExternalInput").ap()
    WT_d = nc.dram_tensor("WT", (D_IN, D_A), bf16, kind="ExternalInput").ap()
    Wb_d = nc.dram_tensor("Wb", (D_A,), f32, kind="ExternalInput").ap()
    UT_d = nc.dram_tensor("UT", (D_A, LC), bf16, kind="ExternalInput").ap()
    V_d = nc.dram_tensor("V", (LC, D_A), f32, kind="ExternalInput").ap()
    Vb_d = nc.dram_tensor("Vb", (LC,), f32, kind="ExternalInput").ap()
    alpha_d = nc.dram_tensor("alpha_out", (B, LC, S), f32, kind="ExternalOutput").ap()
    y_d = nc.dram_tensor("y_out", (B, LC), f32, kind="ExternalOutput").ap()

    with tile.TileContext(nc) as tc:
        with (
            tc.tile_pool(name="consts", bufs=1) as consts,
            tc.tile_pool(name="zpool", bufs=1) as zpool,
            tc.tile_pool(name="xw", bufs=2) as xw,
            tc.tile_pool(name="epool", bufs=2) as epool,
            tc.tile_pool(name="apool", bufs=2) as apool,
            tc.tile_pool(name="spool", bufs=4) as spool,
            tc.tile_pool(name="ypool", bufs=2) as ypool,
            tc.tile_pool(name="psum_z", bufs=2, space="PSUM") as psum_z,
            tc.tile_pool(name="psum_t", bufs=3, space="PSUM") as psum_t,
            tc.tile_pool(name="psum_s", bufs=2, space="PSUM") as psum_s,
            tc.tile_pool(name="psum_m", bufs=1, space="PSUM") as psum_m,
        ):
            # ---- constants ----
            WT_s = consts.tile([P, ND, D_A], bf16)   # [d_in, dc, a]
            nc.sync.dma_start(WT_s[:], WT_d.rearrange("(nd p) a -> p nd a", p=P))
            UT_s = consts.tile([P, NA, LC], bf16)    # [a_in, ac, l]
            nc.sync.dma_start(UT_s[:], UT_d.rearrange("(na p) l -> p na l", p=P))
            V_s = consts.tile([LB, NLB, D_A], f32)   # [l_in, lc, a]
            nc.sync.dma_start(V_s[:], V_d.rearrange("(nl lb) a -> lb nl a", lb=LB))
            with nc.allow_non_contiguous_dma(reason="tiny 1-D bias loads"):
                Wb_s = consts.tile([P, NA], f32)
                nc.gpsimd.dma_start(Wb_s[:], Wb_d.rearrange("(na p) -> p na", p=P))
                Vb_s = consts.tile([LB, NLB], f32)
                nc.gpsimd.dma_start(Vb_s[:], Vb_d.rearrange("(nl lb) -> lb nl", lb=LB))
            ident_f = consts.tile([P, P], f32)
            make_identity(nc, ident_f[:])
            ident_b = consts.tile([P, P], bf16)
            make_identity(nc, ident_b[:])

            for b in range(B):
                # ---------- phase Z: z_T[a, s] and z_nat[s, a] (bf16) ----------
                zT = zpool.tile([P, NA, SP], bf16, tag="zT")
                zN = zpool.tile([P, NS, D_A], bf16, tag="zN")
                for sc in range(NSC):
                    s0 = sc * 512
                    sv = min(512, S - s0)           # 512, or 416 for the last
                    xT = xw.tile([P, ND, 512], bf16)
                    for dc in range(ND):
                        nc.sync.dma_start(
                            out=xT[:, dc, :sv],
                            in_=x_d[b, s0 : s0 + sv, dc * P : (dc + 1) * P],
                            transpose=True,
                        )
                    if sv < 512:
                        nc.gpsimd.memset(xT[:, :, sv:], 0.0)
                    for ao in range(NA):
                        pz = psum_z.tile([P, 512], f32, tag="pz")
                        for dc in range(ND):
                            nc.tensor.matmul(
                                pz[:],
                                WT_s[:, dc, ao * P : (ao + 1) * P],
                                xT[:, dc, :],
                                start=(dc == 0),
                                stop=(dc == ND - 1),
                            )
                        nc.scalar.activation(
                            out=zT[:, ao, s0 : s0 + 512],
                            in_=pz[:],
                            func=Tanh,
                            bias=Wb_s[:, ao : ao + 1],
                        )
                # z_nat from z_T via PE transpose
                for ao in range(NA):
                    for so in range(NS):
                        pt = psum_t.tile([P, P], bf16, tag="tp")
                        nc.tensor.transpose(
                            pt[:], zT[:, ao, so * P : (so + 1) * P], ident_b[:]
                        )
                        nc.vector.tensor_copy(
                            out=zN[:, so, ao * P : (ao + 1) * P], in_=pt[:]
                        )

                # ---------- label phase ----------
                yb = ypool.tile([LB, NLB], f32, tag="yb")
                for lc in range(NLB):
                    l0 = lc * LB
                    expt = epool.tile([LB, SP], f32)
                    sums = spool.tile([LB, NSC], f32)
                    aT = apool.tile([P, NS, LB], bf16)
                    for sc in range(NSC):
                        ps = psum_s.tile([LB, 512], f32, tag="ps")
                        for ao in range(NA):
                            nc.tensor.matmul(
                                ps[:],
                                UT_s[:, ao, l0 : l0 + LB],
                                zT[:, ao, sc * 512 : (sc + 1) * 512],
                                start=(ao == 0),
                                stop=(ao == NA - 1),
                            )
                        if sc == NSC - 1:
                            # pad columns s in [4000, 4096): exp(-30) ~ 1e-13
                            nc.vector.memset(ps[:, S - 7 * 512 :], -30.0)
                        nc.scalar.activation(
                            out=expt[:, sc * 512 : (sc + 1) * 512],
                            in_=ps[:],
                            func=Exp,
                            accum_out=sums[:, sc : sc + 1],
                        )
                        # transpose this chunk of UNNORMALIZED exp right away
                        for so in range(4 * sc, 4 * sc + 4):
                            pt2 = psum_t.tile([P, P], f32, tag="tp")
                            nc.tensor.transpose(
                                pt2[:], expt[:, so * P : (so + 1) * P], ident_f[:]
                            )
                            nc.vector.tensor_copy(out=aT[:, so, :], in_=pt2[:])
                    # m matmul on unnormalized alpha_T
                    pm = psum_m.tile([LB, D_A], f32, tag="pm")
                    for so in range(NS):
                        nc.tensor.matmul(
                            pm[:],
                            aT[:, so, :],
                            zN[:, so, :],
                            start=(so == 0),
                            stop=(so == NS - 1),
                        )
                    tot = spool.tile([LB, 1], f32)
                    nc.vector.reduce_sum(out=tot[:], in_=sums[:], axis=AX)
                    rec = spool.tile([LB, 1], f32)
                    nc.vector.reciprocal(rec[:], tot[:])
                    ytmp = ypool.tile([LB, D_A], f32, tag="ytmp")
                    nc.vector.tensor_mul(ytmp[:], V_s[:, lc, :], pm[:])
                    ysc = spool.tile([LB, 1], f32)
                    nc.vector.reduce_sum(out=ysc[:], in_=ytmp[:], axis=AX)
                    # y = y_unnorm * recip + V_b
                    nc.vector.tensor_scalar(
                        yb[:, lc : lc + 1],
                        ysc[:],
                        rec[:],
                        Vb_s[:, lc : lc + 1],
                        op0=Alu.mult,
                        op1=Alu.add,
                    )
                    # normalized f32 alpha output (off the critical path):
                    # in-place scale AFTER the transposes have consumed expt
                    nc.vector.tensor_scalar_mul(expt[:], expt[:], rec[:])
                    nc.sync.dma_start(out=alpha_d[b, l0 : l0 + LB, :], in_=expt[:, :S])
                with nc.allow_non_contiguous_dma(reason="tiny y store"):
                    nc.gpsimd.dma_start(
                        out=y_d[b].rearrange("(nl lb) -> lb nl", lb=LB), in_=yb[:]
                    )

    nc.compile()
    return nc


def _get_nc():
    if "nc" not in _CACHE:
        _CACHE["nc"] = _build_nc()
    return _CACHE["nc"]


def _make_in_maps(x, W_w, W_b, U_w, V_w, V_b):
    bf = ml_dtypes.bfloat16
    x_bf = np.ascontiguousarray(x, dtype=np.float32).astype(bf)
    WT = np.ascontiguousarray(np.asarray(W_w, np.float32).T).astype(bf)
    Wb = np.ascontiguousarray(W_b, dtype=np.float32)
    U_pad = np.zeros((L_PAD, D_A), np.float32)
    U_pad[:L] = U_w
    UT = np.ascontiguousarray(U_pad.T).astype(bf)          # [D_A, L_PAD]
    V_pad = np.zeros((L_PAD, D_A), np.float32)
    V_pad[:L] = V_w
    Vb_pad = np.zeros((L_PAD,), np.float32)
    Vb_pad[:L] = V_b
    in_maps = []
    for c in range(N_CORES):
        sl = slice(c * LC, (c + 1) * LC)
        in_maps.append(
            {
                "x": x_bf,
                "WT": WT,
                "Wb": Wb,
                "UT": np.ascontiguousarray(UT[:, sl]),
                "V": np.ascontiguousarray(V_pad[sl]),
                "Vb": np.ascontiguousarray(Vb_pad[sl]),
            }
        )
    return in_maps


def run(inputs, trace=False, trace_cores=None):
    """Run on 8 cores; returns ((y, alpha), BassKernelResults)."""
    from concourse.bass_utils import run_bass_kernel_spmd

    nc = _get_nc()
    in_maps = _make_in_maps(**inputs)
    res = run_bass_kernel_spmd(
        nc,
        in_maps,
        core_ids=list(range(N_CORES)),
        trace=trace,
        trace_cores=trace_cores,
    )
    y = np.concatenate([r["y_out"] for r in res.results], axis=1)[:, :L]
    alpha = np.concatenate([r["alpha_out"] for r in res.results], axis=1)[:, :L, :]
    return (np.ascontiguousarray(y), np.ascontiguousarray(alpha)), res


def kernel(x, W_w, W_b, U_w, V_w, V_b):
    (y, alpha), _ = run(
        dict(x=x, W_w=W_w, W_b=W_b, U_w=U_w, V_w=V_w, V_b=V_b), trace=False
    )
    return y, alpha
